# revision 21
# baseline (speedup 1.0000x reference)
"""Multi-head causal attention (B=2, S=2048, E=1024, H=16, Dh=64) on 8 TRN2
NeuronCores.

Sharding: core c handles batch c//4 and the 4 heads [4*(c%4), 4*(c%4)+4).
Each core computes its heads' QKV projections, causal softmax attention, and
a partial output projection (contraction over its 256 d_inner columns).
The host sums the 4 partial outputs per batch (the "all-reduce") and adds
bo_eff = bo + bv @ Wo (the V bias commutes through softmax since the
attention weights sum to 1, so it is folded into the output bias on host).

Device layout notes (PSUM accumulation fp32 everywhere):
  - The Q/K side runs in fp8e4 with DoubleRow matmuls (2 contraction rows
    per partition, 0.5 PE cycles/col): activations and Wq/Wk are shipped
    as fp8 with the embed dim pre-interleaved as [128p, kt, 2j] on the
    host; Q^T/K^T land in SBUF as [128, 2, S] fp8 tiles where head lh
    owns partitions [32*lh, 32*lh+32) and dh index d = 32*j + p%32.  The
    score matmuls then contract dh as [32 partitions x 2 slots] per head
    (explicit tile_position since base partition 96 is rejected by the
    implicit path).  V/PV/O-projection stay bf16: fp8 there fails the
    2e-2 error budget (measured), while fp8 Q/K only costs ~1.4e-2 total
    because score noise is suppressed by the softmax normalization.
  - Scores are computed transposed [k, q]; exp runs on the scalar engine
    (the only engine with an activation unit — after the fp8 changes it
    is the critical resource: ~74us busy vs ~63us PE).  The PV matmul is
    computed as AO[q, dh] = P^T V with keys contracting over partitions,
    with a ones column of V producing the softmax denominators.  AO is
    normalized per-partition on DVE and transposed back to [dh, q] by
    XBAR DMA transposes (PE transposes for the final group).
  - The causal staircase is trimmed at 128-column granularity on the
    diagonal k-tiles; only the leading [128,128] chunk of each needs a
    triangular mask multiply (gpsimd/DVE, which alternate).
  - Q/K/V projection chunks and output-projection tiles are issued as
    "filler" PE work inside the attention kb loops via a credit-based
    pump (exp-time minus attention-PE-time per kb step), so the PE
    stream paces the exp stream.  Groups run (qb,hp) hp0-ascending then
    hp1-ascending; each group's PV-tail flush + normalization +
    transpose is deferred into the next group's kb loop so the exp
    stream never pauses at group boundaries.
  - Host pre-arranges every DRAM tensor into its exact SBUF layout so
    each input DMA is one large-element transfer (few descriptors, one
    HWDGE slot each), ordered so the first exp fires ~7us in.
"""

import numpy as np
import ml_dtypes

import concourse.bass as bass
import concourse.tile as tile
from concourse import bacc, mybir
from concourse.bass_utils import run_bass_kernel_spmd

F32 = mybir.dt.float32
BF16 = mybir.dt.bfloat16
F16 = mybir.dt.float16
F8 = mybir.dt.float8e4

B, S, E = 2, 2048, 1024
H, DH = 16, 64
NCORES = 8
HPC = 4          # heads per core
DL = HPC * DH    # 256: d_inner slice per core
NKT = E // 128   # 8 k-tiles over embed dim
NKT2 = E // 256  # 4 double-k-tiles for fp8 DoubleRow
NST = S // 128   # 16 seq tiles of 128
NQB = S // 512   # 4 q blocks of 512

ExpF = mybir.ActivationFunctionType.Exp
IdF = mybir.ActivationFunctionType.Identity
DR = mybir.MatmulPerfMode.DoubleRow

NPBF16 = ml_dtypes.bfloat16
NPF8 = ml_dtypes.float8_e4m3


def build_nc():
    nc = bacc.Bacc("TRN2", target_bir_lowering=False)

    xq_d = nc.dram_tensor("xq", [128, NKT2, 2, S], F8, kind="ExternalInput")
    xt_d = nc.dram_tensor("xt", [128, NKT, S], BF16, kind="ExternalInput")
    wq_d = nc.dram_tensor("wq", [128, NKT2, 2, DL], F8, kind="ExternalInput")
    wk_d = nc.dram_tensor("wk", [128, NKT2, 2, DL], F8, kind="ExternalInput")
    wv_d = nc.dram_tensor("wv", [128, NKT, DL], BF16, kind="ExternalInput")
    wo_d = nc.dram_tensor("wo", [128, 2, E], BF16, kind="ExternalInput")
    bqk_d = nc.dram_tensor("bqk", [128, 4], F32, kind="ExternalInput")
    tri_d = nc.dram_tensor("tri", [128, 128], BF16, kind="ExternalInput")
    id_d = nc.dram_tensor("ident", [128, 128], BF16, kind="ExternalInput")
    out_d = nc.dram_tensor("out", [E, S], F16, kind="ExternalOutput")

    with tile.TileContext(nc) as tc:
        with (
            tc.tile_pool(name="const", bufs=1) as cp,
            tc.tile_pool(name="work", bufs=1) as wkp,
            tc.tile_pool(name="bpsum", bufs=1, space="PSUM") as bp,
            tc.tile_pool(name="apsum", bufs=1, space="PSUM") as aop,
        ):
            xq = cp.tile([128, NKT2, 2, S], F8, tag="xq", name="xq")
            xt = cp.tile([128, NKT, S], BF16, tag="xt", name="xt")
            wq_sb = cp.tile([128, NKT2, 2, DL], F8, tag="wq_sb", name="wq_sb")
            wk_sb = cp.tile([128, NKT2, 2, DL], F8, tag="wk_sb", name="wk_sb")
            wv_sb = cp.tile([128, NKT, DL], BF16, tag="wv_sb", name="wv_sb")
            wo_sb = cp.tile([128, 2, E], BF16, tag="wo_sb", name="wo_sb")
            bqk = cp.tile([128, 4], F32, tag="bqk", name="bqk")
            tri = cp.tile([128, 128], BF16, tag="tri", name="tri")
            ident = cp.tile([128, 128], BF16, tag="ident", name="ident")
            warmsrc = cp.tile([128, 128], BF16, tag="warmsrc", name="warmsrc")
            qt8 = cp.tile([128, 2, S], F8, tag="qt8", name="qt8")
            kt8 = cp.tile([128, 2, S], F8, tag="kt8", name="kt8")
            v1 = cp.tile([128, NST, HPC * 65], BF16, tag="v1", name="v1")
            ot = [cp.tile([128, S], BF16, tag=f"ot{d}", name=f"ot{d}")
                  for d in range(2)]

            # ---- input DMA stream (ordered for earliest compute start) ----
            nc.sync.dma_start(out=wq_sb[:], in_=wq_d[:])
            nc.sync.dma_start(out=xq[:, :, :, 0:512], in_=xq_d[:, :, :, 0:512])
            nc.sync.dma_start(out=bqk[:], in_=bqk_d[:])
            nc.sync.dma_start(out=wk_sb[:], in_=wk_d[:])
            nc.sync.dma_start(out=xq[:, :, :, 512:1024],
                              in_=xq_d[:, :, :, 512:1024])
            nc.sync.dma_start(out=tri[:], in_=tri_d[:])
            nc.sync.dma_start(out=wv_sb[:], in_=wv_d[:])
            nc.sync.dma_start(out=xt[:, :, 0:256], in_=xt_d[:, :, 0:256])
            nc.sync.dma_start(out=ident[:], in_=id_d[:])
            nc.sync.dma_start(out=xt[:, :, 256:512], in_=xt_d[:, :, 256:512])
            nc.sync.dma_start(out=xq[:, :, :, 1024:1536],
                              in_=xq_d[:, :, :, 1024:1536])
            nc.sync.dma_start(out=xt[:, :, 512:1024], in_=xt_d[:, :, 512:1024])
            nc.sync.dma_start(out=xq[:, :, :, 1536:2048],
                              in_=xq_d[:, :, :, 1536:2048])
            nc.sync.dma_start(out=wo_sb[:], in_=wo_d[:])
            nc.sync.dma_start(out=xt[:, :, 1024:1536],
                              in_=xt_d[:, :, 1024:1536])
            nc.sync.dma_start(out=xt[:, :, 1536:2048],
                              in_=xt_d[:, :, 1536:2048])

            # PE p-state warm-up junk during the DMA lead-in: DVE memset
            # (fastest engine to come up) so the warm matmuls issue early
            nc.vector.memset(warmsrc[:], 0.0)
            # dummy activation: pulls the Exp/Identity table load (1.3us)
            # to the DMA lead-in instead of the first bias-add
            actwarm = wkp.tile([128, 1], BF16, tag="actwarm", bufs=1,
                               name="actwarm")
            nc.scalar.activation(actwarm[:], warmsrc[:, 0:1], ExpF)
            warm = aop.tile([128, 4 * 65], F32, tag="ao0", bufs=1,
                            name="warm")
            for _ in range(37):
                nc.tensor.matmul(warm[:, 0:128], warmsrc[:], warmsrc[:],
                                 start=True, stop=True)

            # ones column of v1 (col 64 of each head's 65-col group)
            nc.gpsimd.memset(
                v1.rearrange("p s (h c) -> p s h c", c=65)[:, :, :, 64:65],
                1.0)

            # ---- projection building blocks ----
            def qk_chunk(mat, j, sb, on_act=False, pool="fil"):
                """qt8/kt8[:, j, sb*512:+512] = fp8 round of W.T X + bias.

                fp8 DoubleRow: contracts 256 embed rows per matmul at 0.5
                PE cycles per output column.  The upfront chunks spread
                across both psum pools and both bias engines (Act is idle
                before the exp stream)."""
                w_sb = wq_sb if mat == 0 else wk_sb
                dst = qt8 if mat == 0 else kt8
                if pool == "st":
                    ps = bp.tile([128, 1024], F32, tag="st", bufs=2,
                                 name="ps")[:, 0:512]
                else:
                    ps = bp.tile([128, 512], F32, tag="fil", bufs=2,
                                 name="ps")
                for kt in range(NKT2):
                    nc.tensor.matmul(
                        ps[:],
                        w_sb[:, kt, :, j * 128:(j + 1) * 128],
                        xq[:, kt, :, sb * 512:(sb + 1) * 512],
                        start=(kt == 0), stop=(kt == NKT2 - 1),
                        perf_mode=DR)
                with nc.allow_low_precision(reason="fp8 round of q/k"):
                    if on_act:
                        nc.scalar.activation(
                            dst[:, j, sb * 512:(sb + 1) * 512], ps[:],
                            IdF, bias=bqk[:, 2 * mat + j:2 * mat + j + 1])
                    else:
                        nc.vector.tensor_scalar_add(
                            dst[:, j, sb * 512:(sb + 1) * 512], ps[:],
                            bqk[:, 2 * mat + j:2 * mat + j + 1])

            def v_chunk(st):
                """v1[:, st, 65h:65h+64] = (X Wv)[st*128:+128, 64h:+64]."""
                ps = bp.tile([128, DL], F32, tag="fil", bufs=2, name="psv")
                for k in range(NKT):
                    nc.tensor.matmul(
                        ps[:],
                        xt[:, k, st * 128:(st + 1) * 128],
                        wv_sb[:, k, :],
                        start=(k == 0), stop=(k == NKT - 1))
                with nc.allow_low_precision(reason="bf16 round of v"):
                    nc.vector.tensor_copy(
                        v1[:, st, :].rearrange("p (h c) -> p h c",
                                               c=65)[:, :, 0:64],
                        ps[:].rearrange("p (h c) -> p h c", c=64))

            # ---- filler queue: PE work pumped into attention kb steps ----
            # Each entry: (tag, pe_ns_estimate, fn).  pump() keeps a credit
            # in ns: attention kb steps add their exp-vs-PE deficit, fillers
            # subtract their cost (credit may go negative and self-balance).
            fillers = []
            state = {"credit": 0.0, "where": ""}
            sched_log = []

            def pump(need_ns):
                state["credit"] = min(state["credit"] + need_ns, 1000.0)
                while fillers and state["credit"] > 0:
                    tag, ns, fn = fillers.pop(0)
                    state["credit"] = max(state["credit"] - ns, -1200.0)
                    sched_log.append(("pump", tag, state["where"]))
                    fn()

            def drain(tags):
                # force-drains run work earlier than the credit schedule
                # would; they do not charge credit.  Selective: only the
                # matching tags run - popping everything in front of the
                # target (the old behaviour) injected multi-filler bursts
                # right where the exp stream needed the PE
                i = 0
                while i < len(fillers):
                    if fillers[i][0] in tags:
                        tag, _, fn = fillers.pop(i)
                        sched_log.append(("drain", tag, state["where"]))
                        fn()
                    else:
                        i += 1

            ob_cur = {}

            def oproj_tile(qb, et, on_act, grp=2):
                """out[et*128:(et+1)*128, qb*512:+512] partial e-tile.

                e-tiles are batched grp-at-a-time into one ob tile and one
                output DMA: the HWDGE slot (~625ns) is the out-path
                serializer, so fewer, bigger DMAs shorten the tail."""
                if et % grp == 0:
                    ob_cur["t"] = wkp.tile([128, grp, 512], F16, tag="ob",
                                           bufs=4, name="ob")
                ob = ob_cur["t"]
                p3 = bp.tile([128, 512], F32, tag="fil", bufs=2, name="p3")
                for d in range(2):
                    nc.tensor.matmul(
                        p3[:],
                        wo_sb[:, d, et * 128:(et + 1) * 128],
                        ot[d][:, qb * 512:(qb + 1) * 512],
                        start=(d == 0), stop=(d == 1))
                with nc.allow_low_precision(reason="fp16 partial out"):
                    if on_act:
                        nc.scalar.copy(out=ob[:, et % grp, :], in_=p3[:])
                    else:
                        nc.vector.tensor_copy(ob[:, et % grp, :], p3[:])
                if et % grp == grp - 1:
                    nc.sync.dma_start(
                        out=out_d[(et - grp + 1) * 128:(et + 1) * 128,
                                  qb * 512:(qb + 1) * 512].rearrange(
                                      "(i p) s -> p i s", p=128),
                        in_=ob[:])

            # ---- attention group: 512-wide q block x 2 heads ----
            # Issues the scores/exp/PV stream for the group; the PV tail
            # flushes, normalization and ot transposes are returned as
            # deferred "closeout" fillers, pumped inside the next group's
            # kb loop so the exp stream never pauses at group boundaries.
            def attn_group(qb, hp, trail=4, norm_on_act=False,
                           final_tail=False, prefetch=()):
                q0 = qb * 512
                nkb = 4 * qb + 4
                aoh = [aop.tile([128, 4 * 65], F32, tag=f"ao{h}", bufs=1,
                                name=f"ao{h}") for h in range(2)]
                pend = []

                def flush_one(in_loop=True):
                    kb, ptt, w, j = pend.pop(0)
                    fkb = kb
                    if in_loop:
                        # PV needs v1[:, kb]; the previous group's closeout
                        # must precede this group's first PV (AO reuse).
                        drain({f"v{kb}", "co"})
                    else:
                        drain({f"v{kb}"})
                    j0 = max(j, 0)
                    for h in range(2):
                        lh = 2 * hp + h
                        for qsub in range(j0, 4):
                            off = h * 512 + (qsub - j0) * 128
                            # one accumulation group per AO bank: start=True
                            # zeroes the whole 2KB zero region, so only the
                            # first matmul into the bank may set it; PSUM
                            # zeroes lazily on each address's first write
                            nc.tensor.matmul(
                                aoh[h][:, qsub * 65:qsub * 65 + 65],
                                ptt[:, off:off + 128],
                                v1[:, kb, lh * 65:(lh + 1) * 65],
                                start=(kb == 0 and qsub == 0),
                                stop=(kb == nkb - 1 and qsub == 3))
                    return fkb

                def tail_qsub(c, in_loop):
                    """Final group only: as soon as qsub c's AO column is
                    complete (its diagonal kb flushed), normalize it,
                    transpose it, and run its slice of the output
                    projection - the kernel tail then ends ~1 qsub (not 1
                    group) after the last exp."""
                    rc2 = wkp.tile([128, 2], F32, tag="rc2", bufs=4,
                                   name="rc2")
                    asb = wkp.tile([128, 128], BF16, tag="aosb", bufs=8,
                                   name="asb")
                    with nc.allow_low_precision(reason="softmax denom"):
                        for h in range(2):
                            nc.vector.reciprocal(
                                rc2[:, h:h + 1],
                                aoh[h][:, c * 65 + 64:c * 65 + 65])
                        for h in range(2):
                            if in_loop:
                                nc.vector.tensor_scalar_mul(
                                    asb[:, h * 64:(h + 1) * 64],
                                    aoh[h][:, c * 65:c * 65 + 64],
                                    rc2[:, h:h + 1])
                            else:
                                nc.scalar.mul(
                                    asb[:, h * 64:(h + 1) * 64],
                                    aoh[h][:, c * 65:c * 65 + 64],
                                    rc2[:, h:h + 1])
                    trp = bp.tile([128, 128], BF16, tag="fil", bufs=2,
                                  name="trp")
                    nc.tensor.transpose(trp[:], asb[:], ident[:])
                    with nc.allow_low_precision(reason="bf16 copy"):
                        nc.vector.tensor_copy(
                            ot[hp][:, q0 + c * 128:q0 + (c + 1) * 128],
                            trp[:])
                    ob2 = wkp.tile([128, 8, 128], F16, tag="ob2", bufs=4,
                                   name="ob2")
                    for half in range(2):
                        p3s = bp.tile([128, 512], F32, tag="fil", bufs=2,
                                      name="p3s")
                        p3v = p3s.rearrange("p (i c) -> p i c", c=128)
                        for i in range(4):
                            et = 4 * half + i
                            for d in range(2):
                                nc.tensor.matmul(
                                    p3v[:, i, :],
                                    wo_sb[:, d, et * 128:(et + 1) * 128],
                                    ot[d][:, q0 + c * 128:
                                          q0 + (c + 1) * 128],
                                    start=(d == 0), stop=(d == 1))
                        with nc.allow_low_precision(reason="fp16 out"):
                            # split the two half-copies across DVE and Act
                            # so they run in parallel on the tail
                            if in_loop or half == 0:
                                nc.vector.tensor_copy(
                                    ob2[:, 4 * half:4 * half + 4, :], p3v)
                            else:
                                nc.scalar.copy(
                                    out=ob2[:, 4 * half:4 * half + 4, :],
                                    in_=p3v)
                    nc.sync.dma_start(
                        out=out_d[:, q0 + c * 128:
                                  q0 + (c + 1) * 128].rearrange(
                                      "(i p) c -> p i c", p=128),
                        in_=ob2[:])

                def issue_scores(kb):
                    """Score matmuls for step kb.  Issued one step AHEAD
                    of the exp/filler block (software pipelining): the PE
                    is in-order, so scores issued after a filler would
                    inherit the filler's latency and stall the exp
                    stream; issued ahead, fillers fill PE idle instead."""
                    j = kb - 4 * qb
                    w = 512 if j < 0 else 512 - 128 * j
                    qs = q0 + (0 if j < 0 else 128 * j)
                    # head h occupies cols [h*512, h*512+w) so every matmul
                    # output stays inside one 2KB PSUM bank
                    st = bp.tile([128, 1024], F32, tag="st", bufs=2,
                                 name="st")
                    for h in range(2):
                        lh = 2 * hp + h
                        nc.tensor.matmul(
                            st[:, h * 512:h * 512 + w],
                            kt8[32 * lh:32 * lh + 32, :,
                                kb * 128:(kb + 1) * 128],
                            qt8[32 * lh:32 * lh + 32, :, qs:q0 + 512],
                            start=True, stop=True,
                            perf_mode=DR, tile_position=(32 * lh, 0))
                    return st, w, j

                # scores read qt8 cols [q0, q0+512) at every kb, and
                # kt8 cols [kb*128, ...) progressively
                drain({f"qkQ0s{qb}", f"qkQ1s{qb}"})
                nxt = issue_scores(0)
                for kb in range(nkb):
                    state["where"] = f"g({qb},{hp})kb{kb}"
                    st, w, j = nxt
                    if kb + 1 < nkb:
                        drain({f"qkK0s{(kb + 1) // 4}",
                               f"qkK1s{(kb + 1) // 4}"})
                        nxt = issue_scores(kb + 1)
                    if kb == 1:
                        # previous group's closeout must be issued before
                        # any of this group's PV matmuls touch AO buffers
                        drain({"co"})
                    ptt = wkp.tile([128, 1024], BF16, tag="pt", bufs=8,
                                   name="pt")
                    nc.scalar.activation(
                        ptt[:].rearrange("p (a b) -> p a b", a=2)[:, :, 0:w],
                        st[:].rearrange("p (a b) -> p a b", a=2)[:, :, 0:w],
                        ExpF, scale=0.125)
                    if j >= 0:
                        for h in range(2):
                            meng = nc.vector if qb <= 1 else (nc.gpsimd if h == 0 else nc.vector)
                            with nc.allow_low_precision(
                                    reason="0/1 mask multiply"):
                                meng.tensor_mul(
                                    ptt[:, h * 512:h * 512 + 128],
                                    ptt[:, h * 512:h * 512 + 128], tri[:])
                    pend.append((kb, ptt, w, j))
                    npv = 2 * (4 - max(j, 0))
                    if len(pend) > trail:
                        fkb = flush_one()
                        if final_tail and fkb >= 4 * qb:
                            tail_qsub(fkb - 4 * qb, in_loop=True)
                    # deficit: exp time minus this kb's attention PE time
                    act_ns = 2 * w * 0.8333 + 250
                    pe_ns = w * 0.4167 + npv * 30
                    pump(act_ns - pe_ns)
                    # backstop: next sb's K chunks must be in flight two kb
                    # steps before the scores that read them
                    nsb = kb // 4 + 1
                    if kb % 4 == 2 and nsb <= qb:
                        drain({f"qkK0s{nsb}", f"qkK1s{nsb}"})

                # pull the next group's first projection chunks now so
                # their DVE copies land before this group's closeout norm
                # clogs the DVE queue at the boundary
                drain(set(prefetch))

                def do_norm():
                    # normalize into [q, dh-pair] SBUF tiles, then XBAR-DMA
                    # transpose each back into ot[hp][:, q block]
                    rc = wkp.tile([128, 8], F32, tag="rcp", bufs=2,
                                  name="rc")
                    for h in range(2):
                        with nc.allow_low_precision(
                                reason="softmax denom recip"):
                            nc.vector.reciprocal(
                                rc[:].rearrange("p (h q) -> p h q",
                                                h=2)[:, h, :].rearrange(
                                                    "p (q c) -> p q c", c=1),
                                aoh[h].rearrange("p (q c) -> p q c",
                                                 c=65)[:, :, 64:65])
                    for qsub in range(4):
                        asb = wkp.tile([128, 128], BF16, tag="aosb", bufs=8,
                                       name="asb")
                        for h in range(2):
                            with nc.allow_low_precision(
                                    reason="bf16 attn out"):
                                if norm_on_act:
                                    # post-exp-stream groups: Act is idle
                                    nc.scalar.mul(
                                        asb[:, h * 64:(h + 1) * 64],
                                        aoh[h][:, qsub * 65:qsub * 65 + 64],
                                        rc[:, h * 4 + qsub:h * 4 + qsub + 1])
                                else:
                                    nc.vector.tensor_scalar_mul(
                                        asb[:, h * 64:(h + 1) * 64],
                                        aoh[h][:, qsub * 65:qsub * 65 + 64],
                                        rc[:, h * 4 + qsub:h * 4 + qsub + 1])
                        # transpose on the PE for every group: the
                        # XBAR-DMA path has ~4-8us latency per closeout,
                        # which kept ot late and parked the downstream
                        # oproj matmuls in the PE wait queue (in-order
                        # SEQ stalls the next group's scores behind them)
                        trp = bp.tile([128, 128], BF16, tag="fil",
                                      bufs=2, name="trp")
                        nc.tensor.transpose(trp[:], asb[:], ident[:])
                        with nc.allow_low_precision(reason="bf16 copy"):
                            nc.vector.tensor_copy(
                                ot[hp][:, q0 + qsub * 128:
                                       q0 + (qsub + 1) * 128], trp[:])

                ntail = len(pend)

                def do_closeout():
                    while pend:
                        fkb = flush_one(in_loop=False)
                        if final_tail and fkb >= 4 * qb:
                            tail_qsub(fkb - 4 * qb, in_loop=False)
                    if not final_tail:
                        do_norm()

                return ("co", ntail * 240.0, do_closeout)

            # ---- schedule ----
            # upfront: just enough projection to start attention (0,0);
            # everything else becomes filler, force-drained on first use
            qk_chunk(0, 0, 0, on_act=True)
            qk_chunk(0, 1, 0)
            qk_chunk(1, 0, 0, on_act=True, pool="st")
            qk_chunk(1, 1, 0, pool="st")

            QK_NS = NKT2 * 512 * 0.5 * 0.4167
            V_NS = 8 * 256 * 0.4167
            OP_NS = 4 * 512 * 0.4167

            def add_qk(sb):
                for mat, nm in ((0, "Q"), (1, "K")):
                    for j in range(2):
                        fillers.append(
                            (f"qk{nm}{j}s{sb}", QK_NS,
                             lambda mat=mat, j=j, sb=sb:
                             qk_chunk(mat, j, sb)))

            def add_v(lo, hi):
                for st in range(lo, hi):
                    fillers.append((f"v{st}", V_NS,
                                    lambda st=st: v_chunk(st)))

            def add_qk1(sb, mat):
                nm = "Q" if mat == 0 else "K"
                for j in range(2):
                    fillers.append(
                        (f"qk{nm}{j}s{sb}", QK_NS,
                         lambda mat=mat, j=j, sb=sb: qk_chunk(mat, j, sb)))

            # deadline order: each entry no later than its force-drain
            # point, and no earlier than its input DMA (a pumped filler
            # whose matmuls park on a DMA semaphore fills the PE wait
            # queue and blocks the in-order SEQ behind it)
            add_qk1(1, 0)
            add_v(0, 2)
            add_qk1(1, 1)
            add_v(2, 4)
            add_qk1(2, 0)
            add_qk1(2, 1)
            add_v(4, 6)
            add_qk1(3, 0)
            add_v(6, 8)
            add_qk1(3, 1)
            add_v(8, 16)

            groups = [(0, 0), (1, 0), (2, 0), (3, 0), (0, 1), (1, 1),
                      (2, 1), (3, 1)]
            for gi, (qb, hp) in enumerate(groups):
                if gi + 1 < len(groups):
                    nqb, nhp = groups[gi + 1]
                    pref = (f"qkQ0s{nqb}", f"qkQ1s{nqb}")
                else:
                    pref = ()
                co = attn_group(qb, hp, trail=2 if gi == 7 else 4,
                                final_tail=(gi == 7), prefetch=pref)
                fillers.insert(0, co)
                if hp == 1 and gi != 7:
                    # past the exp stream's end, Act is idle: alternate the
                    # PSUM->SBUF copies between Act and DVE; the final
                    # group's tiles go out one e-tile at a time so the
                    # last copy+DMA chain is short.  The delay entry holds
                    # the oproj matmuls back until this group's ot XBAR
                    # transposes have landed - the PE is in-order, so an
                    # oproj matmul waiting on a transpose would block the
                    # next group's score matmuls (and stall the exp stream)
                    fillers.append(("dly", 3500.0, lambda: None))
                    grp = 4
                    for et in range(NKT):
                        on_act = False  # Act carries the exp stream
                        fillers.append(
                            (f"op{qb}", OP_NS / 2,
                             lambda qb=qb, et=et, a=on_act, g=grp:
                             oproj_tile(qb, et, a, g)))
            drain({t for t, _, _ in fillers})

    nc.compile()
    nc._sched_log = sched_log
    return nc


_NC = None


def _get_nc():
    global _NC
    if _NC is None:
        _NC = build_nc()
    return _NC


def make_in_maps(inputs, Wq, bq, Wk, bk, Wv, Wo):
    kk = np.arange(128)[:, None]
    qq = np.arange(128)[None, :]
    tri = (qq >= kk).astype(NPBF16)
    ident = np.eye(128, dtype=NPBF16)
    # dl permutation for the fp8 dh-interleave: psum/qt8 partition pi and
    # slot j hold dl = 64*(pi//32) + 32*j + pi%32 (head pi//32, d = 32j+p)
    pi = np.arange(128)
    perm = [64 * (pi // 32) + 32 * j + pi % 32 for j in range(2)]

    def qk_weights(W, g):
        Wg = W[:, g * DL:(g + 1) * DL]                    # [E, 256]
        Wp = np.stack([Wg[:, perm[0]], Wg[:, perm[1]]], 1)  # [E, 2, 128]
        # e = kt*256 + j_e*128 + p  ->  [p, kt, j_e, (j_c 128c)]
        return np.ascontiguousarray(
            Wp.reshape(NKT2, 2, 128, 2, 128).transpose(2, 0, 1, 3, 4)
            .reshape(128, NKT2, 2, DL)).astype(NPF8)

    in_maps = []
    for c in range(NCORES):
        b, g = c // HPC, c % HPC
        sl = slice(g * DL, (g + 1) * DL)
        xtb = np.ascontiguousarray(inputs[b].T)           # [E, S]
        bqk = np.stack([bq[sl][perm[0]], bq[sl][perm[1]],
                        bk[sl][perm[0]], bk[sl][perm[1]]], axis=1)
        in_maps.append({
            "xq": np.ascontiguousarray(
                xtb.reshape(NKT2, 2, 128, S).transpose(2, 0, 1, 3)
            ).astype(NPF8),
            "xt": np.ascontiguousarray(
                xtb.reshape(NKT, 128, S).transpose(1, 0, 2)).astype(NPBF16),
            "wq": qk_weights(Wq, g),
            "wk": qk_weights(Wk, g),
            "wv": np.ascontiguousarray(
                Wv[:, sl].reshape(NKT, 128, DL).transpose(1, 0, 2)
            ).astype(NPBF16),
            "wo": np.ascontiguousarray(
                Wo[sl, :].reshape(2, 128, E).transpose(1, 0, 2)
            ).astype(NPBF16),
            "bqk": np.ascontiguousarray(bqk).astype(np.float32),
            "tri": tri,
            "ident": ident,
        })
    return in_maps


def kernel(inputs, Wq, bq, Wk, bk, Wv, bv, Wo, bo):
    inputs = np.asarray(inputs, np.float32)
    Wq, bq, Wk, bk, Wv, bv, Wo, bo = (
        np.asarray(a, np.float32) for a in (Wq, bq, Wk, bk, Wv, bv, Wo, bo))
    in_maps = make_in_maps(inputs, Wq, bq, Wk, bk, Wv, Wo)
    nc = _get_nc()
    res = run_bass_kernel_spmd(nc, in_maps, list(range(NCORES)))
    bo_eff = bo + bv @ Wo  # V bias commutes through softmax (weights sum to 1)
    outs = []
    for b in range(B):
        acc = res.results[b * HPC]["out"].astype(np.float32)
        for g in range(1, HPC):
            acc = acc + res.results[b * HPC + g]["out"].astype(np.float32)
        outs.append(acc.T + bo_eff)
    return np.stack(outs).astype(np.float32)


# revision 22
# speedup vs baseline: 1.0323x; 1.0323x over previous
"""Multi-head causal attention (B=2, S=2048, E=1024, H=16, Dh=64) on 8 TRN2
NeuronCores.

Sharding: core c handles batch c//4 and the 4 heads [4*(c%4), 4*(c%4)+4).
Each core computes its heads' QKV projections, causal softmax attention, and
a partial output projection (contraction over its 256 d_inner columns).
The host sums the 4 partial outputs per batch (the "all-reduce") and adds
bo_eff = bo + bv @ Wo (the V bias commutes through softmax since the
attention weights sum to 1, so it is folded into the output bias on host).

Device layout notes (PSUM accumulation fp32 everywhere):
  - The Q/K side runs in fp8e4 with DoubleRow matmuls (2 contraction rows
    per partition, 0.5 PE cycles/col): activations and Wq/Wk are shipped
    as fp8 with the embed dim pre-interleaved as [128p, kt, 2j] on the
    host; Q^T/K^T land in SBUF as [128, 2, S] fp8 tiles where head lh
    owns partitions [32*lh, 32*lh+32) and dh index d = 32*j + p%32.  The
    score matmuls then contract dh as [32 partitions x 2 slots] per head
    (explicit tile_position since base partition 96 is rejected by the
    implicit path).  V/PV/O-projection stay bf16: fp8 there fails the
    2e-2 error budget (measured), while fp8 Q/K only costs ~1.4e-2 total
    because score noise is suppressed by the softmax normalization.
  - Scores are computed transposed [k, q]; exp runs on the scalar engine
    (the only engine with an activation unit — after the fp8 changes it
    is the critical resource: ~74us busy vs ~63us PE).  The PV matmul is
    computed as AO[q, dh] = P^T V with keys contracting over partitions,
    with a ones column of V producing the softmax denominators.  AO is
    normalized per-partition on DVE and transposed back to [dh, q] by
    XBAR DMA transposes (PE transposes for the final group).
  - The causal staircase is trimmed at 128-column granularity on the
    diagonal k-tiles; only the leading [128,128] chunk of each needs a
    triangular mask multiply (gpsimd/DVE, which alternate).
  - Q/K/V projection chunks and output-projection tiles are issued as
    "filler" PE work inside the attention kb loops via a credit-based
    pump (exp-time minus attention-PE-time per kb step), so the PE
    stream paces the exp stream.  Groups run (qb,hp) hp0-ascending then
    hp1-ascending; each group's PV-tail flush + normalization +
    transpose is deferred into the next group's kb loop so the exp
    stream never pauses at group boundaries.
  - Host pre-arranges every DRAM tensor into its exact SBUF layout so
    each input DMA is one large-element transfer (few descriptors, one
    HWDGE slot each), ordered so the first exp fires ~7us in.
"""

import numpy as np
import ml_dtypes

import concourse.bass as bass
import concourse.tile as tile
from concourse import bacc, mybir
from concourse.bass_utils import run_bass_kernel_spmd

F32 = mybir.dt.float32
BF16 = mybir.dt.bfloat16
F16 = mybir.dt.float16
F8 = mybir.dt.float8e4

B, S, E = 2, 2048, 1024
H, DH = 16, 64
NCORES = 8
HPC = 4          # heads per core
DL = HPC * DH    # 256: d_inner slice per core
NKT = E // 128   # 8 k-tiles over embed dim
NKT2 = E // 256  # 4 double-k-tiles for fp8 DoubleRow
NST = S // 128   # 16 seq tiles of 128
NQB = S // 512   # 4 q blocks of 512

ExpF = mybir.ActivationFunctionType.Exp
IdF = mybir.ActivationFunctionType.Identity
DR = mybir.MatmulPerfMode.DoubleRow

NPBF16 = ml_dtypes.bfloat16
NPF8 = ml_dtypes.float8_e4m3


def build_nc():
    nc = bacc.Bacc("TRN2", target_bir_lowering=False)

    xq_d = nc.dram_tensor("xq", [128, NKT2, 2, S], F8, kind="ExternalInput")
    xt_d = nc.dram_tensor("xt", [128, NKT, S], BF16, kind="ExternalInput")
    wq_d = nc.dram_tensor("wq", [128, NKT2, 2, DL], F8, kind="ExternalInput")
    wk_d = nc.dram_tensor("wk", [128, NKT2, 2, DL], F8, kind="ExternalInput")
    wv_d = nc.dram_tensor("wv", [128, NKT, DL], BF16, kind="ExternalInput")
    wo_d = nc.dram_tensor("wo", [128, 2, E], BF16, kind="ExternalInput")
    bqk_d = nc.dram_tensor("bqk", [128, 4], F32, kind="ExternalInput")
    tri_d = nc.dram_tensor("tri", [128, 128], BF16, kind="ExternalInput")
    id_d = nc.dram_tensor("ident", [128, 128], BF16, kind="ExternalInput")
    out_d = nc.dram_tensor("out", [E, S], F16, kind="ExternalOutput")

    with tile.TileContext(nc) as tc:
        with (
            tc.tile_pool(name="const", bufs=1) as cp,
            tc.tile_pool(name="work", bufs=1) as wkp,
            tc.tile_pool(name="bpsum", bufs=1, space="PSUM") as bp,
            tc.tile_pool(name="apsum", bufs=1, space="PSUM") as aop,
        ):
            xq = cp.tile([128, NKT2, 2, S], F8, tag="xq", name="xq")
            xt = cp.tile([128, NKT, S], BF16, tag="xt", name="xt")
            wq_sb = cp.tile([128, NKT2, 2, DL], F8, tag="wq_sb", name="wq_sb")
            wk_sb = cp.tile([128, NKT2, 2, DL], F8, tag="wk_sb", name="wk_sb")
            wv_sb = cp.tile([128, NKT, DL], BF16, tag="wv_sb", name="wv_sb")
            wo_sb = cp.tile([128, 2, E], BF16, tag="wo_sb", name="wo_sb")
            bqk = cp.tile([128, 4], F32, tag="bqk", name="bqk")
            tri = cp.tile([128, 128], BF16, tag="tri", name="tri")
            ident = cp.tile([128, 128], BF16, tag="ident", name="ident")
            warmsrc = cp.tile([128, 128], BF16, tag="warmsrc", name="warmsrc")
            qt8 = cp.tile([128, 2, S], F8, tag="qt8", name="qt8")
            kt8 = cp.tile([128, 2, S], F8, tag="kt8", name="kt8")
            v1 = cp.tile([128, NST, HPC * 65], BF16, tag="v1", name="v1")
            ot = [cp.tile([128, S], BF16, tag=f"ot{d}", name=f"ot{d}")
                  for d in range(2)]

            # ---- input DMA stream (ordered for earliest compute start) ----
            nc.sync.dma_start(out=wq_sb[:], in_=wq_d[:])
            nc.sync.dma_start(out=xq[:, :, :, 0:512], in_=xq_d[:, :, :, 0:512])
            nc.sync.dma_start(out=bqk[:], in_=bqk_d[:])
            nc.sync.dma_start(out=wk_sb[:], in_=wk_d[:])
            nc.sync.dma_start(out=xq[:, :, :, 512:1024],
                              in_=xq_d[:, :, :, 512:1024])
            nc.sync.dma_start(out=tri[:], in_=tri_d[:])
            nc.sync.dma_start(out=wv_sb[:], in_=wv_d[:])
            nc.sync.dma_start(out=xt[:, :, 0:256], in_=xt_d[:, :, 0:256])
            nc.sync.dma_start(out=ident[:], in_=id_d[:])
            nc.sync.dma_start(out=xt[:, :, 256:512], in_=xt_d[:, :, 256:512])
            nc.sync.dma_start(out=xq[:, :, :, 1024:1536],
                              in_=xq_d[:, :, :, 1024:1536])
            nc.sync.dma_start(out=xt[:, :, 512:1024], in_=xt_d[:, :, 512:1024])
            nc.sync.dma_start(out=xq[:, :, :, 1536:2048],
                              in_=xq_d[:, :, :, 1536:2048])
            nc.sync.dma_start(out=wo_sb[:], in_=wo_d[:])
            nc.sync.dma_start(out=xt[:, :, 1024:1536],
                              in_=xt_d[:, :, 1024:1536])
            nc.sync.dma_start(out=xt[:, :, 1536:2048],
                              in_=xt_d[:, :, 1536:2048])

            # PE p-state warm-up junk during the DMA lead-in: DVE memset
            # (fastest engine to come up) so the warm matmuls issue early
            nc.vector.memset(warmsrc[:], 0.0)
            # dummy activation: pulls the Exp/Identity table load (1.3us)
            # to the DMA lead-in instead of the first bias-add
            actwarm = wkp.tile([128, 1], BF16, tag="actwarm", bufs=1,
                               name="actwarm")
            nc.scalar.activation(actwarm[:], warmsrc[:, 0:1], ExpF)
            warm = aop.tile([128, 4 * 65], F32, tag="ao0", bufs=1,
                            name="warm")
            for _ in range(37):
                nc.tensor.matmul(warm[:, 0:128], warmsrc[:], warmsrc[:],
                                 start=True, stop=True)

            # ones column of v1 (col 64 of each head's 65-col group)
            nc.gpsimd.memset(
                v1.rearrange("p s (h c) -> p s h c", c=65)[:, :, :, 64:65],
                1.0)

            # ---- projection building blocks ----
            def qk_chunk(mat, j, sb, on_act=False, pool="fil"):
                """qt8/kt8[:, j, sb*512:+512] = fp8 round of W.T X + bias.

                fp8 DoubleRow: contracts 256 embed rows per matmul at 0.5
                PE cycles per output column.  The upfront chunks spread
                across both psum pools and both bias engines (Act is idle
                before the exp stream)."""
                w_sb = wq_sb if mat == 0 else wk_sb
                dst = qt8 if mat == 0 else kt8
                if pool == "st":
                    ps = bp.tile([128, 1024], F32, tag="st", bufs=2,
                                 name="ps")[:, 0:512]
                else:
                    ps = bp.tile([128, 512], F32, tag="fil", bufs=2,
                                 name="ps")
                for kt in range(NKT2):
                    nc.tensor.matmul(
                        ps[:],
                        w_sb[:, kt, :, j * 128:(j + 1) * 128],
                        xq[:, kt, :, sb * 512:(sb + 1) * 512],
                        start=(kt == 0), stop=(kt == NKT2 - 1),
                        perf_mode=DR)
                with nc.allow_low_precision(reason="fp8 round of q/k"):
                    if on_act:
                        nc.scalar.activation(
                            dst[:, j, sb * 512:(sb + 1) * 512], ps[:],
                            IdF, bias=bqk[:, 2 * mat + j:2 * mat + j + 1])
                    else:
                        nc.vector.tensor_scalar_add(
                            dst[:, j, sb * 512:(sb + 1) * 512], ps[:],
                            bqk[:, 2 * mat + j:2 * mat + j + 1])

            def v_chunk(st):
                """v1[:, st, 65h:65h+64] = (X Wv)[st*128:+128, 64h:+64]."""
                ps = bp.tile([128, DL], F32, tag="fil", bufs=2, name="psv")
                for k in range(NKT):
                    nc.tensor.matmul(
                        ps[:],
                        xt[:, k, st * 128:(st + 1) * 128],
                        wv_sb[:, k, :],
                        start=(k == 0), stop=(k == NKT - 1))
                with nc.allow_low_precision(reason="bf16 round of v"):
                    nc.vector.tensor_copy(
                        v1[:, st, :].rearrange("p (h c) -> p h c",
                                               c=65)[:, :, 0:64],
                        ps[:].rearrange("p (h c) -> p h c", c=64))

            # ---- filler queue: PE work pumped into attention kb steps ----
            # Each entry: (tag, pe_ns_estimate, fn).  pump() keeps a credit
            # in ns: attention kb steps add their exp-vs-PE deficit, fillers
            # subtract their cost (credit may go negative and self-balance).
            fillers = []
            state = {"credit": 0.0, "where": ""}
            sched_log = []

            def pump(need_ns):
                state["credit"] = min(state["credit"] + need_ns, 1000.0)
                while fillers and state["credit"] > 0:
                    tag, ns, fn = fillers.pop(0)
                    state["credit"] = max(state["credit"] - ns, -1200.0)
                    sched_log.append(("pump", tag, state["where"]))
                    fn()

            def drain(tags):
                # force-drains run work earlier than the credit schedule
                # would; they do not charge credit.  Selective: only the
                # matching tags run - popping everything in front of the
                # target (the old behaviour) injected multi-filler bursts
                # right where the exp stream needed the PE
                i = 0
                while i < len(fillers):
                    if fillers[i][0] in tags:
                        tag, _, fn = fillers.pop(i)
                        sched_log.append(("drain", tag, state["where"]))
                        fn()
                    else:
                        i += 1

            ob_cur = {}

            def oproj_tile(qb, et, on_act, grp=2):
                """out[et*128:(et+1)*128, qb*512:+512] partial e-tile.

                e-tiles are batched grp-at-a-time into one ob tile and one
                output DMA: the HWDGE slot (~625ns) is the out-path
                serializer, so fewer, bigger DMAs shorten the tail."""
                if et % grp == 0:
                    ob_cur["t"] = wkp.tile([128, grp, 512], F16, tag="ob",
                                           bufs=4, name="ob")
                ob = ob_cur["t"]
                p3 = bp.tile([128, 512], F32, tag="fil", bufs=2, name="p3")
                for d in range(2):
                    nc.tensor.matmul(
                        p3[:],
                        wo_sb[:, d, et * 128:(et + 1) * 128],
                        ot[d][:, qb * 512:(qb + 1) * 512],
                        start=(d == 0), stop=(d == 1))
                with nc.allow_low_precision(reason="fp16 partial out"):
                    if on_act:
                        nc.scalar.copy(out=ob[:, et % grp, :], in_=p3[:])
                    else:
                        nc.vector.tensor_copy(ob[:, et % grp, :], p3[:])
                if et % grp == grp - 1:
                    nc.sync.dma_start(
                        out=out_d[(et - grp + 1) * 128:(et + 1) * 128,
                                  qb * 512:(qb + 1) * 512].rearrange(
                                      "(i p) s -> p i s", p=128),
                        in_=ob[:])

            # ---- attention group: 512-wide q block x 2 heads ----
            # Issues the scores/exp/PV stream for the group; the PV tail
            # flushes, normalization and ot transposes are returned as
            # deferred "closeout" fillers, pumped inside the next group's
            # kb loop so the exp stream never pauses at group boundaries.
            def attn_group(qb, hp, trail=4, norm_on_act=False,
                           final_tail=False, prefetch=()):
                q0 = qb * 512
                nkb = 4 * qb + 4
                aoh = [aop.tile([128, 4 * 65], F32, tag=f"ao{h}", bufs=1,
                                name=f"ao{h}") for h in range(2)]
                pend = []

                def flush_one(in_loop=True):
                    kb, ptt, w, j = pend.pop(0)
                    fkb = kb
                    if in_loop:
                        # PV needs v1[:, kb]; the previous group's closeout
                        # must precede this group's first PV (AO reuse).
                        drain({f"v{kb}", "co"})
                    else:
                        drain({f"v{kb}"})
                    j0 = max(j, 0)
                    for h in range(2):
                        lh = 2 * hp + h
                        for qsub in range(j0, 4):
                            off = h * 512 + (qsub - j0) * 128
                            # one accumulation group per AO bank: start=True
                            # zeroes the whole 2KB zero region, so only the
                            # first matmul into the bank may set it; PSUM
                            # zeroes lazily on each address's first write
                            nc.tensor.matmul(
                                aoh[h][:, qsub * 65:qsub * 65 + 65],
                                ptt[:, off:off + 128],
                                v1[:, kb, lh * 65:(lh + 1) * 65],
                                start=(kb == 0 and qsub == 0),
                                stop=(kb == nkb - 1 and qsub == 3))
                    return fkb

                def tail_qsub(c, in_loop):
                    """Final group only: as soon as qsub c's AO column is
                    complete (its diagonal kb flushed), normalize it,
                    transpose it, and run its slice of the output
                    projection - the kernel tail then ends ~1 qsub (not 1
                    group) after the last exp."""
                    rc2 = wkp.tile([128, 2], F32, tag="rc2", bufs=4,
                                   name="rc2")
                    asb = wkp.tile([128, 128], BF16, tag="aosb", bufs=8,
                                   name="asb")
                    with nc.allow_low_precision(reason="softmax denom"):
                        for h in range(2):
                            nc.vector.reciprocal(
                                rc2[:, h:h + 1],
                                aoh[h][:, c * 65 + 64:c * 65 + 65])
                        for h in range(2):
                            if in_loop:
                                nc.vector.tensor_scalar_mul(
                                    asb[:, h * 64:(h + 1) * 64],
                                    aoh[h][:, c * 65:c * 65 + 64],
                                    rc2[:, h:h + 1])
                            else:
                                nc.scalar.mul(
                                    asb[:, h * 64:(h + 1) * 64],
                                    aoh[h][:, c * 65:c * 65 + 64],
                                    rc2[:, h:h + 1])
                    trp = bp.tile([128, 128], BF16, tag="fil", bufs=2,
                                  name="trp")
                    nc.tensor.transpose(trp[:], asb[:], ident[:])
                    with nc.allow_low_precision(reason="bf16 copy"):
                        nc.vector.tensor_copy(
                            ot[hp][:, q0 + c * 128:q0 + (c + 1) * 128],
                            trp[:])
                    ob2 = wkp.tile([128, 8, 128], F16, tag="ob2", bufs=4,
                                   name="ob2")
                    for half in range(2):
                        p3s = bp.tile([128, 512], F32, tag="fil", bufs=2,
                                      name="p3s")
                        p3v = p3s.rearrange("p (i c) -> p i c", c=128)
                        for i in range(4):
                            et = 4 * half + i
                            for d in range(2):
                                nc.tensor.matmul(
                                    p3v[:, i, :],
                                    wo_sb[:, d, et * 128:(et + 1) * 128],
                                    ot[d][:, q0 + c * 128:
                                          q0 + (c + 1) * 128],
                                    start=(d == 0), stop=(d == 1))
                        with nc.allow_low_precision(reason="fp16 out"):
                            # split the two half-copies across DVE and Act
                            # so they run in parallel on the tail
                            if in_loop or half == 0:
                                nc.vector.tensor_copy(
                                    ob2[:, 4 * half:4 * half + 4, :], p3v)
                            else:
                                nc.scalar.copy(
                                    out=ob2[:, 4 * half:4 * half + 4, :],
                                    in_=p3v)
                    nc.sync.dma_start(
                        out=out_d[:, q0 + c * 128:
                                  q0 + (c + 1) * 128].rearrange(
                                      "(i p) c -> p i c", p=128),
                        in_=ob2[:])

                def issue_scores(kb):
                    """Score matmuls for step kb.  Issued one step AHEAD
                    of the exp/filler block (software pipelining): the PE
                    is in-order, so scores issued after a filler would
                    inherit the filler's latency and stall the exp
                    stream; issued ahead, fillers fill PE idle instead."""
                    j = kb - 4 * qb
                    w = 512 if j < 0 else 512 - 128 * j
                    qs = q0 + (0 if j < 0 else 128 * j)
                    # head h occupies cols [h*512, h*512+w) so every matmul
                    # output stays inside one 2KB PSUM bank
                    st = bp.tile([128, 1024], F32, tag="st", bufs=2,
                                 name="st")
                    for h in range(2):
                        lh = 2 * hp + h
                        nc.tensor.matmul(
                            st[:, h * 512:h * 512 + w],
                            kt8[32 * lh:32 * lh + 32, :,
                                kb * 128:(kb + 1) * 128],
                            qt8[32 * lh:32 * lh + 32, :, qs:q0 + 512],
                            start=True, stop=True,
                            perf_mode=DR, tile_position=(32 * lh, 0))
                    return st, w, j

                # scores read qt8 cols [q0, q0+512) at every kb, and
                # kt8 cols [kb*128, ...) progressively
                drain({f"qkQ0s{qb}", f"qkQ1s{qb}"})
                nxt = issue_scores(0)
                for kb in range(nkb):
                    state["where"] = f"g({qb},{hp})kb{kb}"
                    st, w, j = nxt
                    if kb + 1 < nkb:
                        drain({f"qkK0s{(kb + 1) // 4}",
                               f"qkK1s{(kb + 1) // 4}"})
                        nxt = issue_scores(kb + 1)
                    ptt = wkp.tile([128, 1024], BF16, tag="pt", bufs=8,
                                   name="pt")
                    nc.scalar.activation(
                        ptt[:].rearrange("p (a b) -> p a b", a=2)[:, :, 0:w],
                        st[:].rearrange("p (a b) -> p a b", a=2)[:, :, 0:w],
                        ExpF, scale=0.125)
                    if j >= 0:
                        for h in range(2):
                            meng = nc.vector if qb <= 1 else (nc.gpsimd if h == 0 else nc.vector)
                            with nc.allow_low_precision(
                                    reason="0/1 mask multiply"):
                                meng.tensor_mul(
                                    ptt[:, h * 512:h * 512 + 128],
                                    ptt[:, h * 512:h * 512 + 128], tri[:])
                    pend.append((kb, ptt, w, j))
                    npv = 2 * (4 - max(j, 0))
                    if len(pend) > trail:
                        fkb = flush_one()
                        if final_tail and fkb >= 4 * qb:
                            tail_qsub(fkb - 4 * qb, in_loop=True)
                    # deficit: exp time minus this kb's attention PE time
                    act_ns = 2 * w * 0.8333 + 250
                    pe_ns = w * 0.4167 + npv * 30
                    pump(act_ns - pe_ns)
                    # backstop: next sb's K chunks must be in flight two kb
                    # steps before the scores that read them
                    nsb = kb // 4 + 1
                    if kb % 4 == 2 and nsb <= qb:
                        drain({f"qkK0s{nsb}", f"qkK1s{nsb}"})

                # pull the next group's first projection chunks now so
                # their DVE copies land before this group's closeout norm
                # clogs the DVE queue at the boundary
                drain(set(prefetch))

                def do_norm():
                    # normalize into [q, dh-pair] SBUF tiles, then XBAR-DMA
                    # transpose each back into ot[hp][:, q block]
                    rc = wkp.tile([128, 8], F32, tag="rcp", bufs=2,
                                  name="rc")
                    for h in range(2):
                        with nc.allow_low_precision(
                                reason="softmax denom recip"):
                            nc.vector.reciprocal(
                                rc[:].rearrange("p (h q) -> p h q",
                                                h=2)[:, h, :].rearrange(
                                                    "p (q c) -> p q c", c=1),
                                aoh[h].rearrange("p (q c) -> p q c",
                                                 c=65)[:, :, 64:65])
                    for qsub in range(4):
                        asb = wkp.tile([128, 128], BF16, tag="aosb", bufs=8,
                                       name="asb")
                        for h in range(2):
                            with nc.allow_low_precision(
                                    reason="bf16 attn out"):
                                if norm_on_act:
                                    # post-exp-stream groups: Act is idle
                                    nc.scalar.mul(
                                        asb[:, h * 64:(h + 1) * 64],
                                        aoh[h][:, qsub * 65:qsub * 65 + 64],
                                        rc[:, h * 4 + qsub:h * 4 + qsub + 1])
                                else:
                                    nc.vector.tensor_scalar_mul(
                                        asb[:, h * 64:(h + 1) * 64],
                                        aoh[h][:, qsub * 65:qsub * 65 + 64],
                                        rc[:, h * 4 + qsub:h * 4 + qsub + 1])
                        # transpose on the PE for every group: the
                        # XBAR-DMA path has ~4-8us latency per closeout,
                        # which kept ot late and parked the downstream
                        # oproj matmuls in the PE wait queue (in-order
                        # SEQ stalls the next group's scores behind them)
                        trp = bp.tile([128, 128], BF16, tag="fil",
                                      bufs=2, name="trp")
                        nc.tensor.transpose(trp[:], asb[:], ident[:])
                        with nc.allow_low_precision(reason="bf16 copy"):
                            nc.vector.tensor_copy(
                                ot[hp][:, q0 + qsub * 128:
                                       q0 + (qsub + 1) * 128], trp[:])

                def co_flush():
                    fkb = flush_one(in_loop=False)
                    if final_tail and fkb >= 4 * qb:
                        tail_qsub(fkb - 4 * qb, in_loop=False)

                # split the closeout into one filler per pending flush plus
                # the norm: an atomic closeout (4 flushes + their forced
                # v-chunks + norm) is a ~4.5us PE block that stalls the
                # next group's exp stream wherever the pump drops it
                cos = [("co", 450.0, co_flush) for _ in pend]
                if not final_tail:
                    cos.append(("co", 800.0, do_norm))
                return cos

            # ---- schedule ----
            # upfront: just enough projection to start attention (0,0);
            # everything else becomes filler, force-drained on first use
            qk_chunk(0, 0, 0, on_act=True)
            qk_chunk(0, 1, 0)
            qk_chunk(1, 0, 0, on_act=True, pool="st")
            qk_chunk(1, 1, 0, pool="st")

            QK_NS = NKT2 * 512 * 0.5 * 0.4167
            V_NS = 8 * 256 * 0.4167
            OP_NS = 4 * 512 * 0.4167

            def add_qk(sb):
                for mat, nm in ((0, "Q"), (1, "K")):
                    for j in range(2):
                        fillers.append(
                            (f"qk{nm}{j}s{sb}", QK_NS,
                             lambda mat=mat, j=j, sb=sb:
                             qk_chunk(mat, j, sb)))

            def add_v(lo, hi):
                for st in range(lo, hi):
                    fillers.append((f"v{st}", V_NS,
                                    lambda st=st: v_chunk(st)))

            def add_qk1(sb, mat):
                nm = "Q" if mat == 0 else "K"
                for j in range(2):
                    fillers.append(
                        (f"qk{nm}{j}s{sb}", QK_NS,
                         lambda mat=mat, j=j, sb=sb: qk_chunk(mat, j, sb)))

            # deadline order: each entry no later than its force-drain
            # point, and no earlier than its input DMA (a pumped filler
            # whose matmuls park on a DMA semaphore fills the PE wait
            # queue and blocks the in-order SEQ behind it)
            add_qk1(1, 0)
            add_qk1(1, 1)
            add_v(0, 4)
            add_qk1(2, 0)
            add_qk1(2, 1)
            add_v(4, 8)
            add_qk1(3, 0)
            add_qk1(3, 1)
            add_v(8, 16)

            groups = [(0, 0), (1, 0), (2, 0), (3, 0), (0, 1), (1, 1),
                      (2, 1), (3, 1)]
            for gi, (qb, hp) in enumerate(groups):
                if gi + 1 < len(groups):
                    nqb, nhp = groups[gi + 1]
                    pref = (f"qkQ0s{nqb}", f"qkQ1s{nqb}")
                else:
                    pref = ()
                cos = attn_group(qb, hp, trail=2 if gi == 7 else 4,
                                 final_tail=(gi == 7), prefetch=pref)
                fillers[0:0] = cos
                if hp == 1 and gi != 7:
                    # past the exp stream's end, Act is idle: alternate the
                    # PSUM->SBUF copies between Act and DVE; the final
                    # group's tiles go out one e-tile at a time so the
                    # last copy+DMA chain is short.  The delay entry holds
                    # the oproj matmuls back until this group's ot XBAR
                    # transposes have landed - the PE is in-order, so an
                    # oproj matmul waiting on a transpose would block the
                    # next group's score matmuls (and stall the exp stream)
                    fillers.append(("dly", 3500.0, lambda: None))
                    grp = 4
                    for et in range(NKT):
                        on_act = False  # Act carries the exp stream
                        fillers.append(
                            (f"op{qb}", OP_NS / 2,
                             lambda qb=qb, et=et, a=on_act, g=grp:
                             oproj_tile(qb, et, a, g)))
            drain({t for t, _, _ in fillers})

    nc.compile()
    nc._sched_log = sched_log
    return nc


_NC = None


def _get_nc():
    global _NC
    if _NC is None:
        _NC = build_nc()
    return _NC


def make_in_maps(inputs, Wq, bq, Wk, bk, Wv, Wo):
    kk = np.arange(128)[:, None]
    qq = np.arange(128)[None, :]
    tri = (qq >= kk).astype(NPBF16)
    ident = np.eye(128, dtype=NPBF16)
    # dl permutation for the fp8 dh-interleave: psum/qt8 partition pi and
    # slot j hold dl = 64*(pi//32) + 32*j + pi%32 (head pi//32, d = 32j+p)
    pi = np.arange(128)
    perm = [64 * (pi // 32) + 32 * j + pi % 32 for j in range(2)]

    def qk_weights(W, g):
        Wg = W[:, g * DL:(g + 1) * DL]                    # [E, 256]
        Wp = np.stack([Wg[:, perm[0]], Wg[:, perm[1]]], 1)  # [E, 2, 128]
        # e = kt*256 + j_e*128 + p  ->  [p, kt, j_e, (j_c 128c)]
        return np.ascontiguousarray(
            Wp.reshape(NKT2, 2, 128, 2, 128).transpose(2, 0, 1, 3, 4)
            .reshape(128, NKT2, 2, DL)).astype(NPF8)

    in_maps = []
    for c in range(NCORES):
        b, g = c // HPC, c % HPC
        sl = slice(g * DL, (g + 1) * DL)
        xtb = np.ascontiguousarray(inputs[b].T)           # [E, S]
        bqk = np.stack([bq[sl][perm[0]], bq[sl][perm[1]],
                        bk[sl][perm[0]], bk[sl][perm[1]]], axis=1)
        in_maps.append({
            "xq": np.ascontiguousarray(
                xtb.reshape(NKT2, 2, 128, S).transpose(2, 0, 1, 3)
            ).astype(NPF8),
            "xt": np.ascontiguousarray(
                xtb.reshape(NKT, 128, S).transpose(1, 0, 2)).astype(NPBF16),
            "wq": qk_weights(Wq, g),
            "wk": qk_weights(Wk, g),
            "wv": np.ascontiguousarray(
                Wv[:, sl].reshape(NKT, 128, DL).transpose(1, 0, 2)
            ).astype(NPBF16),
            "wo": np.ascontiguousarray(
                Wo[sl, :].reshape(2, 128, E).transpose(1, 0, 2)
            ).astype(NPBF16),
            "bqk": np.ascontiguousarray(bqk).astype(np.float32),
            "tri": tri,
            "ident": ident,
        })
    return in_maps


def kernel(inputs, Wq, bq, Wk, bk, Wv, bv, Wo, bo):
    inputs = np.asarray(inputs, np.float32)
    Wq, bq, Wk, bk, Wv, bv, Wo, bo = (
        np.asarray(a, np.float32) for a in (Wq, bq, Wk, bk, Wv, bv, Wo, bo))
    in_maps = make_in_maps(inputs, Wq, bq, Wk, bk, Wv, Wo)
    nc = _get_nc()
    res = run_bass_kernel_spmd(nc, in_maps, list(range(NCORES)))
    bo_eff = bo + bv @ Wo  # V bias commutes through softmax (weights sum to 1)
    outs = []
    for b in range(B):
        acc = res.results[b * HPC]["out"].astype(np.float32)
        for g in range(1, HPC):
            acc = acc + res.results[b * HPC + g]["out"].astype(np.float32)
        outs.append(acc.T + bo_eff)
    return np.stack(outs).astype(np.float32)
